# revision 36
# baseline (speedup 1.0000x reference)
"""Trainium2 Bass kernel for nn_Attention_39934605918652.

res[b] = W0 @ x0[b] + sum_{n=1..N-1} W2 @ tanh(W1a @ x0[b] + W1b @ x[b,n])

Key algebraic optimization: W2 does not depend on n, so
    sum_n W2 @ tanh(...) = W2 @ (sum_n tanh(...))
which removes the second big matmul (only a [B,H]x[H,F] remains).

Sharding: data-parallel over batch B=128 across 8 cores (16 batches/core),
weights replicated. No collectives.

Matmul operands are 16-bit (xi/W1b in bf16; x0/W1a/W2/W0/S in fp16, whose
10-bit mantissa matches tf32) streaming at 1 PE cycle/row; PSUM
accumulation is fp32. Measured end-to-end rel err vs a float64 oracle:
~4.4e-4. (KB_DT=f32r gives ~1.9e-4 at ~+10% time; KB_NO_F32R=1 gives
full-fp32 matmuls, ~3e-7, at ~2.5x time.)

Per-core timeline: inputs stream ~22us (HBM-bound; compute overlaps),
256 N=512 matmuls dominate the PE (~55us warm at 2.4GHz), bias+tanh via
per-batch ACT `activation(bias=h0)` calls into bf16 scratch, segmented
free-dim reduce_sum on the DVE, then a 12-matmul epilogue computes
res = W0 x0 + W2 S with batch as the PE's M dim.

Device layout (per core), f-major so the contraction dim sits on SBUF
partitions:
  xiT   [F=512, BL*256]  columns grouped 256 per batch (255 real + 1 zero pad)
  x0T   [128, 4*BL]      host-packed f-chunks side by side
  w1bT  [F=512, H=1024]  (= W1[:, F:].T)
  w1aT  [F=512, H=1024]  (= W1[:, :F].T)
  w2T   [H=1024, F=512]  (= W2.T)
  w0T   [F=512, F=512]   (= W0.T)
Output res [BL=16, F=512] per core (batch-major); host concatenates.
"""

import os
import numpy as np
from contextlib import ExitStack

import concourse.bass as bass
import concourse.tile as tile
from concourse import bacc, mybir
from concourse.bass_utils import run_bass_kernel_spmd

N_CORES = 8
B, N, F, H = 128, 256, 512, 1024
BL = B // N_CORES          # 16 batches per core
NI = N - 1                 # 255 real columns per batch
NP = 256                   # padded columns per batch
NF = F // 128              # 4 f-chunks
NH = H // 128              # 8 h-tiles
QUADS = BL // 4            # 4 batch-quads; per quad psum tile [128, 4*256]

F32 = mybir.dt.float32
F32R = mybir.dt.float32r
BF16 = mybir.dt.bfloat16
F16 = mybir.dt.float16


def _dtypes():
    """(dtype for xi/w1b, dtype for x0/w1a/w2/w0/S)."""
    if not USE_F32R:
        return F32, F32
    if KB_DT == "bf16all":
        return BF16, BF16
    if KB_DT == "bf16xi":
        return BF16, F32R
    if KB_DT == "f16":
        # fp16 has the same 10-bit mantissa as tf32 at half the bytes;
        # all values here are O(1) so the 5-bit exponent is plenty.
        return BF16, F16
    return F32R, F32R

USE_F32R = os.environ.get("KB_NO_F32R", "") == ""
# KB_DT: "f16" (default: xi/w1b bf16, rest fp16), "bf16xi" (xi/w1b bf16,
# rest f32r), "f32r", "bf16all". 16-bit operands halve DMA bytes and the
# PE's SBUF read bandwidth (which otherwise contends with concurrent DMA
# writes) at a small precision cost.
KB_DT = os.environ.get("KB_DT", "f16")
# How many of the 32 quad-tiles take the ACT consumer path (bias+tanh in
# per-batch activation calls, freeing the PSUM slot without a DVE hop);
# the rest take the DVE broadcast-bias path. 32 measured fastest.
N_ACT_PATH = int(os.environ.get("KB_NACT", "32"))


def _build_kernel():
    nc = bacc.Bacc(
        "TRN2", target_bir_lowering=False, debug=False, num_devices=N_CORES
    )

    XIDT, WDT = _dtypes()
    xiT = nc.dram_tensor("xiT", [F, BL * NP], XIDT, kind="ExternalInput").ap()
    x0T = nc.dram_tensor("x0T", [128, NF * BL], WDT, kind="ExternalInput").ap()
    w1bT = nc.dram_tensor("w1bT", [F, H], XIDT, kind="ExternalInput").ap()
    w1aT = nc.dram_tensor("w1aT", [F, H], WDT, kind="ExternalInput").ap()
    w2T = nc.dram_tensor("w2T", [H, F], WDT, kind="ExternalInput").ap()
    w0T = nc.dram_tensor("w0T", [F, F], WDT, kind="ExternalInput").ap()
    res = nc.dram_tensor("res", [BL, F], F32, kind="ExternalOutput").ap()

    with tile.TileContext(nc) as tc:
        with ExitStack() as ctx:
            _kernel_body(ctx, tc, xiT, x0T, w1bT, w1aT, w2T, w0T, res)

    nc.compile()
    return nc


def _kernel_body(ctx, tc, xiT, x0T, w1bT, w1aT, w2T, w0T, res):
    nc = tc.nc
    Tanh = mybir.ActivationFunctionType.Tanh
    XIDT, WDT = _dtypes()

    wpool = ctx.enter_context(tc.tile_pool(name="weights", bufs=1))

    def load(name, dram, rows, width, dt):
        tiles = []
        for c in range(rows // 128):
            t = wpool.tile([128, width], dt, tag=f"{name}_{c}", name=f"{name}_{c}")
            nc.sync.dma_start(t[:], dram[c * 128 : (c + 1) * 128, :])
            tiles.append(t)
        return tiles

    # DMA issue order = first-need order. x0 arrives host-packed [128, NF*BL].
    x0_all = wpool.tile([128, NF * BL], WDT, tag="x0", name="x0_all")
    nc.sync.dma_start(x0_all[:], x0T[:])
    x0_sb = [x0_all[:, f * BL : (f + 1) * BL] for f in range(NF)]
    w1a_sb = load("w1a", w1aT, F, H, WDT)
    w1b_sb = load("w1b", w1bT, F, H, XIDT)
    # xi as 8 half-column tiles, all c0 halves DMA'd before any c1 half:
    # wave-major compute below starts on c0 while c1 still streams.
    HC = BL * NP // 2
    xi_sb = [[None, None] for _ in range(NF)]
    for c in range(2):
        for f in range(NF):
            t = wpool.tile([128, HC], XIDT, tag=f"xi_{f}_{c}", name=f"xi_{f}_{c}")
            nc.sync.dma_start(
                t[:], xiT[f * 128 : (f + 1) * 128, c * HC : (c + 1) * HC]
            )
            xi_sb[f][c] = t
    w2_sb = load("w2", w2T, H, F, WDT)
    w0_sb = load("w0", w0T, F, F, WDT)

    h0_sb = [
        wpool.tile([128, BL], F32, tag=f"h0_{h}", name=f"h0_{h}")
        for h in range(NH)
    ]
    S_sb = [
        wpool.tile([128, BL], WDT, tag=f"S_{h}", name=f"S_{h}")
        for h in range(NH)
    ]

    # One PSUM pool; every tile shares the tag so slots recycle.
    # Slot size = max tile = [128, 4*NP] f32 = 2 banks; 4 bufs = 8 banks.
    ppool = ctx.enter_context(tc.tile_pool(name="ps", bufs=4, space="PSUM"))
    itpool = ctx.enter_context(tc.tile_pool(name="it", bufs=4))

    # ---- Phase 0: PE warm-up during the DMA lead-in ----
    # The PE sits idle for the first ~15us while inputs stream from HBM;
    # HAM then holds it at 1.2GHz for the first ~3.4us of real work and
    # re-throttles after every stall. A stream of dummy matmuls on zeros
    # (no DMA dependency) keeps the activity monitor warm so real matmuls
    # issue at 2.4GHz from the start.
    # Plain fp32 dummies (4 cyc/row -> ~850ns each warm): ~18 cover the
    # ~15us DMA window. (f32r here trips a walrus ISA check on the memset.)
    warm_n = int(os.environ.get("KB_WARM", "0"))
    if warm_n:
        wz = wpool.tile([128, 512], F32, tag="warmz", name="warmz")
        nc.vector.memset(wz[:], 0.0)
        pw = ppool.tile([128, 512], F32, tag="ps", name="pwarm")
        for _ in range(warm_n):
            nc.tensor.matmul(pw[:], wz[:, :128], wz[:], start=True, stop=True)

    # ---- Phase 0b: preload the tanh ACT table during the DMA lead-in
    # (first ACTIVATE otherwise pays the ~2.7us table load mid-kernel).
    tiny = wpool.tile([128, 1], F32, tag="tiny", name="tiny")
    nc.vector.memset(tiny[:], 0.0)
    nc.scalar.activation(tiny[:], tiny[:], Tanh)

    # ---- Phase 1: h0[h, b] = sum_f W1a[h, f] * x0[b, f] ----
    for h in range(NH):
        ph = ppool.tile([128, BL], F32, tag="ps", name=f"ph0_{h}")
        for f in range(NF):
            nc.tensor.matmul(
                ph[:],
                w1a_sb[f][:, h * 128 : (h + 1) * 128],
                x0_sb[f],
                start=(f == 0),
                stop=(f == NF - 1),
            )
        nc.vector.tensor_copy(h0_sb[h][:], ph[:])

    # ---- Phase 2: hi matmul + bias + tanh + segmented reduce ----
    def act_path(idx):
        return (idx * N_ACT_PATH) // 32 != ((idx + 1) * N_ACT_PATH) // 32

    _consume_counter = [0]

    def consume(h, q, pb):
        idx = _consume_counter[0]
        _consume_counter[0] += 1
        it = itpool.tile([128, 4 * NP], BF16, tag="it", name=f"it_{h}_{q}")
        view = it[:].rearrange("p (b n) -> p b n", b=4)[:, :, :NI]
        if act_path(idx):
            # path A: bias+tanh fused on ACT, one call per batch
            for bl in range(4):
                b = q * 4 + bl
                nc.scalar.activation(
                    it[:, bl * NP : bl * NP + NI],
                    pb[:, bl * NP : bl * NP + NI],
                    Tanh,
                    bias=h0_sb[h][:, b : b + 1],
                )
        else:
            # path B: one DVE op adds h0 to all 4 batch blocks (in1
            # broadcast along n with stride 0), then one big tanh on ACT
            pbv = pb[:].rearrange("p (b n) -> p b n", b=4)[:, :, :NI]
            h0b = (
                h0_sb[h][:, q * 4 : (q + 1) * 4]
                .unsqueeze(2)
                .broadcast_to([128, 4, NI])
            )
            nc.vector.tensor_add(view, pbv, h0b)
            nc.scalar.activation(view, view, Tanh)
        with nc.allow_low_precision(
            reason="S accumulated in 16-bit to feed the 16-bit output matmul"
        ):
            nc.vector.reduce_sum(
                S_sb[h][:, q * 4 : (q + 1) * 4],
                view,
                axis=mybir.AxisListType.X,
            )

    def mm_tile(pb, h, q, f, wave):
        lhsT = w1b_sb[f][:, h * 128 : (h + 1) * 128]
        for bk in range(2):
            cols = slice(
                (q % 2) * 4 * NP + bk * 512,
                (q % 2) * 4 * NP + (bk + 1) * 512,
            )
            nc.tensor.matmul(
                pb[:, bk * 512 : (bk + 1) * 512],
                lhsT,
                xi_sb[f][wave][:, cols],
                start=(f == 0),
                stop=(f == NF - 1),
            )

    for wave in range(QUADS // 2):
        qs = (2 * wave, 2 * wave + 1)
        for h in range(NH):
            pbs = {
                q: ppool.tile([128, 4 * NP], F32, tag="ps", name=f"pb_{h}_{q}")
                for q in qs
            }
            for f in range(NF):
                for q in qs:
                    mm_tile(pbs[q], h, q, f, wave)
            for q in qs:
                consume(h, q, pbs[q])

    # ---- Phase 3 (flipped): res[b, g] = sum_h S[h,b] W2T[h,g]
    #                                   + sum_f x0T[f,b] W0T[f,g] ----
    # b (=16) is the PE's M dim; N=512 streams. 12 matmuls total.
    # W0 term first: it has no S dependency, so it runs while the last
    # wave's consumers are still producing S.
    po = ppool.tile([BL, F], F32, tag="ps", name="po")
    for f in range(NF):
        nc.tensor.matmul(
            po[:], x0_sb[f], w0_sb[f][:], start=(f == 0), stop=False
        )
    for h in range(NH):
        nc.tensor.matmul(
            po[:], S_sb[h][:], w2_sb[h][:], start=False, stop=(h == NH - 1)
        )
    rt = itpool.tile([BL, F], F32, tag="rt", name="rt")
    nc.vector.tensor_copy(rt[:], po[:])
    nc.sync.dma_start(res[:], rt[:])


_NC_CACHE = {}


def _get_nc():
    key = ("v19", USE_F32R, KB_DT, N_ACT_PATH, os.environ.get("KB_WARM", "0"))
    if key not in _NC_CACHE:
        _NC_CACHE[key] = _build_kernel()
    return _NC_CACHE[key]


def _np_dt(dt):
    import ml_dtypes
    if dt == BF16:
        return ml_dtypes.bfloat16
    if dt == F16:
        return np.float16
    return np.float32


def _make_in_maps(x, W1, W2, W0):
    xidt, wdt = _dtypes()
    np_xi, np_w = _np_dt(xidt), _np_dt(wdt)
    x = np.ascontiguousarray(np.asarray(x, dtype=np.float32))
    W1 = np.asarray(W1, dtype=np.float32)
    W2 = np.asarray(W2, dtype=np.float32)
    W0 = np.asarray(W0, dtype=np.float32)

    w1aT = np.ascontiguousarray(W1[:, :F].T).astype(np_w)   # [F, H]
    w1bT = np.ascontiguousarray(W1[:, F:].T).astype(np_xi)  # [F, H]
    w2T = np.ascontiguousarray(W2.T).astype(np_w)           # [H, F]
    w0T = np.ascontiguousarray(W0.T).astype(np_w)           # [F, F]

    in_maps = []
    for i in range(N_CORES):
        xc = x[i * BL : (i + 1) * BL]               # [BL, N, F]
        # packed [128, NF*BL]: row p, block f holds x0T[f*128+p, :]
        x0T = np.ascontiguousarray(
            xc[:, 0, :].T.reshape(NF, 128, BL).transpose(1, 0, 2).reshape(128, NF * BL)
        ).astype(np_w)
        pad = np.zeros((BL, NP, F), dtype=np.float32)
        pad[:, :NI, :] = xc[:, 1:, :]
        xiT = np.ascontiguousarray(pad.reshape(BL * NP, F).T).astype(np_xi)
        in_maps.append(
            {
                "xiT": xiT,
                "x0T": x0T,
                "w1bT": w1bT,
                "w1aT": w1aT,
                "w2T": w2T,
                "w0T": w0T,
            }
        )
    return in_maps


def _gather(results):
    out = np.empty((B, F), dtype=np.float32)
    for i in range(N_CORES):
        out[i * BL : (i + 1) * BL] = results[i]["res"]
    return out


def kernel(x, W1, W2, W0):
    nc = _get_nc()
    in_maps = _make_in_maps(x, W1, W2, W0)
    res = run_bass_kernel_spmd(nc, in_maps, list(range(N_CORES)))
    return _gather(res.results)


def kernel_profiled(x, W1, W2, W0, **trace_kwargs):
    """Like kernel() but with NTFF profiling; returns (out, exec_time_ns)."""
    nc = _get_nc()
    in_maps = _make_in_maps(x, W1, W2, W0)
    res = run_bass_kernel_spmd(
        nc, in_maps, list(range(N_CORES)), trace=True, **trace_kwargs
    )
    return _gather(res.results), res.exec_time_ns

